# revision 39
# baseline (speedup 1.0000x reference)
"""Causal multi-head attention (B=1, S=4096, D=768, H=12, d_head=64) on 8
Trainium2 NeuronCores.

Sharding: tensor-parallel over heads. 12 heads are mapped onto 16 head-slots
(2 per core); the 4 leftover heads are duplicated onto two slots of the same
core with their W_out rows pre-scaled by 0.5, keeping the SPMD program
uniform across cores. The host sums the 8 partial row-parallel
out-projection outputs and adds b_out.

All matmul operands are bf16 (PSUM accumulates f32): x arrives
host-pre-transposed as xT [768, 4096] bf16 so no on-device transposes or
casts are needed to feed the Q/K/V projections. Q/K/V for one query tile
share a single 3-bank PSUM tile. exp runs on 3-block groups
(ACT instruction overhead amortized) with scale=1/8 and bias=-40 (cancels in
normalization, keeps unnormalized weights in range). Softmax denominators
come free as a ones-column appended to V in the PV stationary; their
reciprocal uses the fast approx DVE op. The out-projection streams per query
tile and the partial output is written in bf16 (halves output DMA).
"""

import sys

sys.path.insert(0, "/opt/trn_rl_repo")

import ml_dtypes
import numpy as np

import concourse.bass as bass
import concourse.tile as tile
from concourse import bacc, mybir
from concourse.bass_utils import run_bass_kernel_spmd

S = 4096
D = 768
HD = 64
P = 128
KC = D // P  # 6 contraction chunks for the projections
QT_W = 512  # query-tile width (one psum bank of f32)
NQT = S // QT_W  # 8 query tiles
NKB = S // P  # 32 key blocks
GRP = 3  # score blocks per exp group (3 psum banks)

F32 = mybir.dt.float32
BF16 = mybir.dt.bfloat16
AF = mybir.ActivationFunctionType
EXP_BIAS = -40.0

SLOTS = [(0, 1), (2, 3), (4, 5), (6, 7), (8, 8), (9, 9), (10, 10), (11, 11)]
SCALES = [(1.0, 1.0)] * 4 + [(0.5, 0.5)] * 4

DEBUG = False  # add DRAM dumps of intermediates (qT/k2/vA/cT)
_CACHED_NC = None


def build_nc():
    nc = bacc.Bacc("TRN2", target_bir_lowering=False, debug=False, num_devices=8)

    xt_d = nc.declare_dram_parameter("xt", [D, S], BF16, isOutput=False)
    wq_d = nc.declare_dram_parameter("wq", [D, P], BF16, isOutput=False)
    wk_d = nc.declare_dram_parameter("wk", [D, P], BF16, isOutput=False)
    wv_d = nc.declare_dram_parameter("wv", [D, P], BF16, isOutput=False)
    wo_d = nc.declare_dram_parameter("wo", [P, D], BF16, isOutput=False)
    mask_d = nc.declare_dram_parameter("mask", [P, P], F32, isOutput=False)
    ident_d = nc.declare_dram_parameter("ident", [P, P], BF16, isOutput=False)
    out_d = nc.declare_dram_parameter("out", [S, D], BF16, isOutput=True)
    if DEBUG:
        dbg_x_d = nc.declare_dram_parameter("dbg_x", [P, KC, S], BF16, isOutput=True)
        dbg_q_d = nc.declare_dram_parameter("dbg_q", [P, S], BF16, isOutput=True)
        dbg_k0_d = nc.declare_dram_parameter("dbg_k0", [P, S], BF16, isOutput=True)
        dbg_va_d = nc.declare_dram_parameter(
            "dbg_va", [P, NKB, 2 * P], BF16, isOutput=True
        )
        dbg_ct_d = nc.declare_dram_parameter("dbg_ct", [P, S], BF16, isOutput=True)
        dbg_sc_d = nc.declare_dram_parameter("dbg_sc", [P, QT_W], F32, isOutput=True)
        dbg_p_d = nc.declare_dram_parameter("dbg_p", [P, QT_W], BF16, isOutput=True)
        dbg_cx_d = nc.declare_dram_parameter("dbg_cx", [P, QT_W], F32, isOutput=True)

    with tile.TileContext(nc) as tc:
        with (
            tc.tile_pool(name="const", bufs=1) as const,
            tc.tile_pool(name="big", bufs=1) as big,
        ):
            # ---- constants / persistent SBUF ----
            mask_s = const.tile([P, P], F32)
            nc.sync.dma_start(mask_s[:], mask_d[:])
            ident = const.tile([P, P], BF16)
            nc.sync.dma_start(ident[:], ident_d[:])
            w_r = const.tile([P, KC, 3 * P], BF16)
            nc.sync.dma_start(w_r[:, :, 0:P], wq_d.rearrange("(c p) m -> p c m", p=P))
            nc.sync.dma_start(
                w_r[:, :, P : 2 * P], wk_d.rearrange("(c p) m -> p c m", p=P)
            )
            nc.sync.dma_start(
                w_r[:, :, 2 * P : 3 * P], wv_d.rearrange("(c p) m -> p c m", p=P)
            )
            wo_r = const.tile([P, D], BF16)
            nc.sync.dma_start(wo_r[:], wo_d[:])

            warm = const.tile([P, QT_W], BF16)
            nc.gpsimd.memset(warm[:], 0.5)
            ebias = const.tile([P, 1], F32)
            nc.gpsimd.memset(ebias[:], EXP_BIAS)

            xT = big.tile([P, KC, S], BF16)  # d-on-partitions x, streamed in
            qT = big.tile([P, S], BF16)  # rows 0:64 slot A, 64:128 slot B
            # zero-padded per-slot keys: partial (64-row / 65-col) stationary
            # tiles run the PE at half rate, so keep every stationary 128x128
            k2 = [big.tile([P, S], BF16, name=f"k2_{i}") for i in (0, 1)]
            # vA[key, kb, slot*128+j]: j 0 = ones (denominator lands on psum
            # partition 0, where the custom recip op needs it; psum partition
            # bases must be 32-aligned so ctx values go to partitions 64:128),
            # j 1:64 = 0, j 64:128 = V_slot
            vA = big.tile([P, NKB, 2 * P], BF16)
            cT = big.tile([P, S], BF16)  # normalized ctx: 0:64 A, 64:128 B

            nc.gpsimd.memset(k2[0][HD:P, :], 0.0)
            nc.gpsimd.memset(k2[1][0:HD, :], 0.0)
            for slot in (0, 1):
                nc.gpsimd.memset(vA[:, :, slot * P], 1.0)
                nc.gpsimd.memset(vA[:, :, slot * P + 1 : slot * P + HD], 0.0)

            # one psum pool for the whole kernel: tag "sc" = 2 x 3 banks
            # (proj Q/K/V triple, score groups), tag "ctx" = 2 x 1 bank
            # (V-transposes, ctx accumulators, out-proj halves) -> 8 banks,
            # no phase barrier between projection and attention
            with (
                tc.tile_pool(name="scp", bufs=2, space="PSUM") as scp,
                tc.tile_pool(name="stg", bufs=3) as stg,
                tc.tile_pool(name="pt", bufs=6) as pt,
                tc.tile_pool(name="sm", bufs=6) as sm,
            ):
                # warm the PE p-state while the first x chunk streams in
                for wi in range(16):
                    wps = scp.tile([P, GRP * QT_W], F32, name="sc", tag="sc")
                    nc.tensor.matmul(
                        wps[:, 0:QT_W], ident[:], warm[:], start=True, stop=True
                    )
                def project(t):
                    nc.sync.dma_start(
                        xT[:, :, t * QT_W : (t + 1) * QT_W],
                        xt_d.rearrange("(c p) s -> p c s", p=P)[
                            :, :, t * QT_W : (t + 1) * QT_W
                        ],
                    )
                    pj = scp.tile([P, GRP * QT_W], F32, name="sc", tag="sc")
                    for j in range(3):  # Q, K, V share one 3-bank psum tile
                        for c in range(KC):
                            nc.tensor.matmul(
                                pj[:, j * QT_W : (j + 1) * QT_W],
                                w_r[:, c, j * P : (j + 1) * P],
                                xT[:, c, t * QT_W : (t + 1) * QT_W],
                                start=(c == 0),
                                stop=(c == KC - 1),
                            )
                    # split the four psum->sbuf casts across ACT (idle during
                    # the projection phase) and DVE so the pj ring slot frees
                    # at PE rate
                    nc.scalar.copy(qT[:, t * QT_W : (t + 1) * QT_W], pj[:, 0:QT_W])
                    nc.vector.tensor_copy(
                        k2[0][0:HD, t * QT_W : (t + 1) * QT_W],
                        pj[0:HD, QT_W : 2 * QT_W],
                    )
                    nc.vector.tensor_copy(
                        k2[1][HD:P, t * QT_W : (t + 1) * QT_W],
                        pj[HD:P, QT_W : 2 * QT_W],
                    )
                    vt = stg.tile([P, QT_W], BF16, name="vt", tag="vt")
                    nc.scalar.copy(vt[:], pj[:, 2 * QT_W : 3 * QT_W])
                    for b in range(QT_W // P):
                        kb = t * 4 + b
                        tp = scp.tile([P, P], BF16, name="ctx", tag="ctx")
                        nc.tensor.transpose(
                            tp[:], vt[:, b * P : (b + 1) * P], ident[:]
                        )
                        nc.vector.tensor_copy(vA[:, kb, HD:P], tp[:, 0:HD])
                        nc.vector.tensor_copy(vA[:, kb, P + HD : 2 * P], tp[:, HD:P])

                # ---- attention + out-projection ----
                def attend(t, slot):
                    """Head slot 0/1: qdims at rows [slot*64, slot*64+64)."""
                    off = slot * HD
                    nkb = 4 * (t + 1)
                    ctx = scp.tile([P, QT_W], F32, name="ctx", tag="ctx")
                    q_mv = qT[:, t * QT_W : (t + 1) * QT_W]
                    for g0 in range(0, nkb, GRP):
                        kbs = range(g0, min(g0 + GRP, nkb))
                        gw = len(kbs) * QT_W
                        sc = scp.tile([P, GRP * QT_W], F32, name="sc", tag="sc")
                        for i, kb in enumerate(kbs):
                            # diagonal blocks only need columns r0: onward; the
                            # skipped psum cols hold stale-but-finite garbage
                            # that exp maps to junk p values PV never reads
                            r0 = max(0, kb * P - t * QT_W)
                            nc.tensor.matmul(
                                sc[:, i * QT_W + r0 : (i + 1) * QT_W],
                                k2[slot][:, kb * P : (kb + 1) * P],
                                q_mv[:, r0:QT_W],
                                start=True,
                                stop=True,
                            )
                        for i, kb in enumerate(kbs):
                            r = kb * P - t * QT_W
                            if r >= 0:
                                nc.vector.tensor_tensor(
                                    sc[:, i * QT_W + r : i * QT_W + r + P],
                                    sc[:, i * QT_W + r : i * QT_W + r + P],
                                    mask_s[:],
                                    mybir.AluOpType.add,
                                )
                        if DEBUG and t == 0 and slot == 0 and g0 == 0:
                            dsc = sm.tile([P, QT_W], F32, name="dsc", tag="dsc")
                            nc.vector.tensor_copy(dsc[:], sc[:, 0:QT_W])
                            nc.sync.dma_start(dbg_sc_d[:], dsc[:])
                        # columns left of a diagonal block's r0 were never
                        # written and are never read by PV; skip them in the
                        # exp. One instruction per group normally, but split
                        # per-block when the trimmed columns outweigh the
                        # ~140ns per-instruction ACT overhead (diag-heavy
                        # final groups).
                        l0 = max(0, kbs[0] * P - t * QT_W)
                        r0s = [max(0, kb * P - t * QT_W) for kb in kbs]
                        cost_single = (gw - l0) * 0.833 + 140
                        cost_split = sum((QT_W - r) * 0.833 + 140 for r in r0s)
                        p_t = pt.tile([P, GRP * QT_W], BF16, name="ptile")
                        if cost_split < cost_single:
                            for i, r in enumerate(r0s):
                                nc.scalar.activation(
                                    p_t[:, i * QT_W + r : (i + 1) * QT_W],
                                    sc[:, i * QT_W + r : (i + 1) * QT_W],
                                    AF.Exp,
                                    scale=0.125,
                                    bias=ebias[:],
                                )
                        else:
                            nc.scalar.activation(
                                p_t[:, l0:gw],
                                sc[:, l0:gw],
                                AF.Exp,
                                scale=0.125,
                                bias=ebias[:],
                            )
                        if DEBUG and t == 0 and slot == 0 and g0 == 0:
                            nc.sync.dma_start(dbg_p_d[:], p_t[:, 0:QT_W])
                        for i, kb in enumerate(kbs):
                            r0 = max(0, kb * P - t * QT_W)
                            nc.tensor.matmul(
                                ctx[:, r0:QT_W],
                                vA[:, kb, slot * P : (slot + 1) * P],
                                p_t[:, i * QT_W + r0 : (i + 1) * QT_W],
                                start=(kb == 0),
                                stop=(kb == nkb - 1),
                            )
                    if DEBUG and t == 0 and slot == 0:
                        dcx = sm.tile([P, QT_W], F32, name="dcx", tag="dcx")
                        nc.vector.tensor_copy(dcx[:], ctx[:])
                        nc.sync.dma_start(dbg_cx_d[:], dcx[:])
                    rr = sm.tile([1, QT_W], F32, name="rr", tag="rr")
                    nc.vector.reciprocal_approx_fast(rr[:], ctx[0:1, :])
                    lb = sm.tile([HD, QT_W], F32, name="lb", tag="lb")
                    nc.gpsimd.partition_broadcast(lb[:], rr[0:1, :])
                    nc.vector.tensor_tensor(
                        cT[off : off + HD, t * QT_W : (t + 1) * QT_W],
                        ctx[HD:P, :],
                        lb[:],
                        mybir.AluOpType.mult,
                    )

                def outproj(t):
                    # each 384-wide f32 half gets its own 1-bank psum tile
                    for b in range(QT_W // P):
                        st = t * 4 + b
                        o_stage = sm.tile([P, D], BF16, name="o_stage", tag="ost")
                        for nch in range(2):
                            po = scp.tile([P, QT_W], F32, name="ctx", tag="ctx")
                            nc.tensor.matmul(
                                po[:, 0 : D // 2],
                                cT[:, st * P : (st + 1) * P],
                                wo_r[:, nch * (D // 2) : (nch + 1) * (D // 2)],
                                start=True,
                                stop=True,
                            )
                            nc.vector.tensor_copy(
                                o_stage[:, nch * (D // 2) : (nch + 1) * (D // 2)],
                                po[:, 0 : D // 2],
                            )
                        nc.sync.dma_start(out_d[st * P : (st + 1) * P, :], o_stage[:])

                # projection first (interleaving proj into the attention loop
                # thrashes the 2-deep sc psum ring and throttles the PE);
                # out-projection trails attention by one tile so its cT
                # dependency (recip/broadcast/mult) never stalls the PE queue
                for t in range(NQT):
                    project(t)
                for t in range(NQT):
                    attend(t, 0)
                    attend(t, 1)
                    if t > 0:
                        outproj(t - 1)
                outproj(NQT - 1)

                if DEBUG:
                    nc.sync.dma_start(dbg_x_d[:], xT[:])
                    nc.sync.dma_start(dbg_q_d[:], qT[:])
                    nc.sync.dma_start(dbg_k0_d[:], k2[0][:])
                    nc.sync.dma_start(dbg_va_d[:], vA[:])
                    nc.sync.dma_start(dbg_ct_d[:], cT[:])

    nc.compile()
    return nc


def _host_inputs(x, W_query, W_key, W_value, W_out):
    mask = np.where(
        np.arange(P)[:, None] <= np.arange(P)[None, :], 0.0, -1e30
    ).astype(np.float32)
    ident = np.eye(P, dtype=ml_dtypes.bfloat16)
    xt = np.ascontiguousarray(x.T).astype(ml_dtypes.bfloat16)
    in_maps = []
    for core in range(8):
        ha, hb = SLOTS[core]
        sa, sb = SCALES[core]
        ca, cb = slice(ha * HD, (ha + 1) * HD), slice(hb * HD, (hb + 1) * HD)
        in_maps.append(
            {
                "xt": xt,
                "wq": np.ascontiguousarray(
                    np.concatenate([W_query[:, ca], W_query[:, cb]], axis=1)
                ).astype(ml_dtypes.bfloat16),
                "wk": np.ascontiguousarray(
                    np.concatenate([W_key[:, ca], W_key[:, cb]], axis=1)
                ).astype(ml_dtypes.bfloat16),
                "wv": np.ascontiguousarray(
                    np.concatenate([W_value[:, ca], W_value[:, cb]], axis=1)
                ).astype(ml_dtypes.bfloat16),
                "wo": np.ascontiguousarray(
                    np.concatenate([W_out[ca, :] * sa, W_out[cb, :] * sb], axis=0)
                ).astype(ml_dtypes.bfloat16),
                "mask": mask,
                "ident": ident,
            }
        )
    return in_maps


def run(x, W_query, W_key, W_value, W_out, b_out, trace=False):
    global _CACHED_NC
    if _CACHED_NC is None:
        _CACHED_NC = build_nc()
    nc = _CACHED_NC
    in_maps = _host_inputs(x, W_query, W_key, W_value, W_out)
    res = run_bass_kernel_spmd(nc, in_maps, core_ids=list(range(8)), trace=trace)
    out = np.zeros((S, D), dtype=np.float32)
    for core in range(8):
        out += res.results[core]["out"].astype(np.float32)
    out += b_out[None, :].astype(np.float32)
    return out, res


def kernel(x, W_query, W_key, W_value, W_out, b_out):
    x2 = np.asarray(x, dtype=np.float32).reshape(S, D)
    out, _ = run(
        x2,
        np.asarray(W_query, np.float32),
        np.asarray(W_key, np.float32),
        np.asarray(W_value, np.float32),
        np.asarray(W_out, np.float32),
        np.asarray(b_out, np.float32),
    )
    return out.reshape(1, S, D)


# revision 44
# speedup vs baseline: 1.2558x; 1.2558x over previous
"""Causal multi-head attention (B=1, S=4096, D=768, H=12, d_head=64) on 8
Trainium2 NeuronCores.

Sharding: tensor-parallel over heads. 12 heads are mapped onto 16 head-slots
(2 per core); the 4 leftover heads are duplicated onto two slots of the same
core with their W_out rows pre-scaled by 0.5, keeping the SPMD program
uniform across cores. The host sums the 8 partial row-parallel
out-projection outputs and adds b_out.

All matmul operands are bf16 (PSUM accumulates f32): x arrives
host-pre-transposed as xT [768, 4096] bf16 so no on-device transposes or
casts are needed to feed the Q/K/V projections. Q/K/V for one query tile
share a single 3-bank PSUM tile. exp runs on 3-block groups
(ACT instruction overhead amortized) with scale=1/8 and bias=-40 (cancels in
normalization, keeps unnormalized weights in range). Softmax denominators
come free as a ones-column appended to V in the PV stationary; their
reciprocal uses the fast approx DVE op. The out-projection streams per query
tile and the partial output is written in bf16 (halves output DMA).
"""

import sys

sys.path.insert(0, "/opt/trn_rl_repo")

import ml_dtypes
import numpy as np

import concourse.bass as bass
import concourse.tile as tile
from concourse import bacc, mybir
from concourse.bass_utils import run_bass_kernel_spmd

S = 4096
D = 768
HD = 64
P = 128
KC = D // P  # 6 contraction chunks for the projections
QT_W = 512  # query-tile width (one psum bank of f32)
NQT = S // QT_W  # 8 query tiles
NKB = S // P  # 32 key blocks
GRP = 3  # score blocks per exp group (3 psum banks)

F32 = mybir.dt.float32
BF16 = mybir.dt.bfloat16
AF = mybir.ActivationFunctionType
EXP_BIAS = -40.0

SLOTS = [(0, 1), (2, 3), (4, 5), (6, 7), (8, 8), (9, 9), (10, 10), (11, 11)]
SCALES = [(1.0, 1.0)] * 4 + [(0.5, 0.5)] * 4

DEBUG = False  # add DRAM dumps of intermediates (qT/k2/vA/cT)
_CACHED_NC = None


def build_nc():
    nc = bacc.Bacc("TRN2", target_bir_lowering=False, debug=False, num_devices=8)

    xt_d = nc.declare_dram_parameter("xt", [D, S], BF16, isOutput=False)
    wq_d = nc.declare_dram_parameter("wq", [D, P], BF16, isOutput=False)
    wk_d = nc.declare_dram_parameter("wk", [D, P], BF16, isOutput=False)
    wv_d = nc.declare_dram_parameter("wv", [D, P], BF16, isOutput=False)
    wo_d = nc.declare_dram_parameter("wo", [P, D], BF16, isOutput=False)
    mask_d = nc.declare_dram_parameter("mask", [P, P], BF16, isOutput=False)
    ident_d = nc.declare_dram_parameter("ident", [P, P], BF16, isOutput=False)
    out_d = nc.declare_dram_parameter("out", [S, D], BF16, isOutput=True)
    if DEBUG:
        dbg_x_d = nc.declare_dram_parameter("dbg_x", [P, KC, S], BF16, isOutput=True)
        dbg_q_d = nc.declare_dram_parameter("dbg_q", [P, S], BF16, isOutput=True)
        dbg_k0_d = nc.declare_dram_parameter("dbg_k0", [P, S], BF16, isOutput=True)
        dbg_va_d = nc.declare_dram_parameter(
            "dbg_va", [P, NKB, 2 * P], BF16, isOutput=True
        )
        dbg_ct_d = nc.declare_dram_parameter("dbg_ct", [P, S], BF16, isOutput=True)
        dbg_sc_d = nc.declare_dram_parameter("dbg_sc", [P, QT_W], F32, isOutput=True)
        dbg_p_d = nc.declare_dram_parameter("dbg_p", [P, QT_W], BF16, isOutput=True)
        dbg_cx_d = nc.declare_dram_parameter("dbg_cx", [P, QT_W], F32, isOutput=True)

    with tile.TileContext(nc) as tc:
        with (
            tc.tile_pool(name="const", bufs=1) as const,
            tc.tile_pool(name="big", bufs=1) as big,
        ):
            # ---- constants / persistent SBUF ----
            mask_s = const.tile([P, P], BF16)
            nc.sync.dma_start(mask_s[:], mask_d[:])
            ident = const.tile([P, P], BF16)
            nc.sync.dma_start(ident[:], ident_d[:])
            w_r = const.tile([P, KC, 3 * P], BF16)
            nc.sync.dma_start(w_r[:, :, 0:P], wq_d.rearrange("(c p) m -> p c m", p=P))
            nc.sync.dma_start(
                w_r[:, :, P : 2 * P], wk_d.rearrange("(c p) m -> p c m", p=P)
            )
            nc.sync.dma_start(
                w_r[:, :, 2 * P : 3 * P], wv_d.rearrange("(c p) m -> p c m", p=P)
            )
            wo_r = const.tile([P, D], BF16)
            nc.sync.dma_start(wo_r[:], wo_d[:])

            warm = const.tile([P, QT_W], BF16)
            nc.gpsimd.memset(warm[:], 0.5)
            ebias = const.tile([P, 1], F32)
            nc.gpsimd.memset(ebias[:], EXP_BIAS)

            xT = big.tile([P, KC, S], BF16)  # d-on-partitions x, streamed in
            qT = big.tile([P, S], BF16)  # rows 0:64 slot A, 64:128 slot B
            # zero-padded per-slot keys: partial (64-row / 65-col) stationary
            # tiles run the PE at half rate, so keep every stationary 128x128
            k2 = [big.tile([P, S], BF16, name=f"k2_{i}") for i in (0, 1)]
            # vA[key, kb, slot*128+j]: j 0 = ones (denominator lands on psum
            # partition 0, where the custom recip op needs it; psum partition
            # bases must be 32-aligned so ctx values go to partitions 64:128),
            # j 1:64 = 0, j 64:128 = V_slot
            vA = big.tile([P, NKB, 2 * P], BF16)
            cT = big.tile([P, S], BF16)  # normalized ctx: 0:64 A, 64:128 B

            nc.gpsimd.memset(k2[0][HD:P, :], 0.0)
            nc.gpsimd.memset(k2[1][0:HD, :], 0.0)
            for slot in (0, 1):
                nc.gpsimd.memset(vA[:, :, slot * P], 1.0)
                nc.gpsimd.memset(vA[:, :, slot * P + 1 : slot * P + HD], 0.0)

            # one psum pool for the whole kernel: tag "sc" = 2 x 3 banks
            # (proj Q/K/V triple, score groups), tag "ctx" = 2 x 1 bank
            # (V-transposes, ctx accumulators, out-proj halves) -> 8 banks,
            # no phase barrier between projection and attention
            with (
                tc.tile_pool(name="scp", bufs=2, space="PSUM") as scp,
                tc.tile_pool(name="stg", bufs=2) as stg,
                tc.tile_pool(name="pt", bufs=4) as pt,
                tc.tile_pool(name="sm", bufs=4) as sm,
            ):
                # warm the PE p-state while the first x chunk streams in
                for wi in range(16):
                    wps = scp.tile([P, GRP * QT_W], F32, name="sc", tag="sc")
                    nc.tensor.matmul(
                        wps[:, 0:QT_W], ident[:], warm[:], start=True, stop=True
                    )
                def project(t):
                    nc.sync.dma_start(
                        xT[:, :, t * QT_W : (t + 1) * QT_W],
                        xt_d.rearrange("(c p) s -> p c s", p=P)[
                            :, :, t * QT_W : (t + 1) * QT_W
                        ],
                    )
                    pj = scp.tile([P, GRP * QT_W], F32, name="sc", tag="sc")
                    for j in range(3):  # Q, K, V share one 3-bank psum tile
                        for c in range(KC):
                            nc.tensor.matmul(
                                pj[:, j * QT_W : (j + 1) * QT_W],
                                w_r[:, c, j * P : (j + 1) * P],
                                xT[:, c, t * QT_W : (t + 1) * QT_W],
                                start=(c == 0),
                                stop=(c == KC - 1),
                            )
                    # split the four psum->sbuf casts across ACT (idle during
                    # the projection phase) and DVE so the pj ring slot frees
                    # at PE rate
                    nc.scalar.copy(qT[:, t * QT_W : (t + 1) * QT_W], pj[:, 0:QT_W])
                    nc.vector.tensor_copy(
                        k2[0][0:HD, t * QT_W : (t + 1) * QT_W],
                        pj[0:HD, QT_W : 2 * QT_W],
                    )
                    nc.vector.tensor_copy(
                        k2[1][HD:P, t * QT_W : (t + 1) * QT_W],
                        pj[HD:P, QT_W : 2 * QT_W],
                    )
                    vt = stg.tile([P, QT_W], BF16, name="vt", tag="vt")
                    nc.scalar.copy(vt[:], pj[:, 2 * QT_W : 3 * QT_W])
                    for b in range(QT_W // P):
                        kb = t * 4 + b
                        tp = scp.tile([P, P], BF16, name="ctx", tag="ctx")
                        nc.tensor.transpose(
                            tp[:], vt[:, b * P : (b + 1) * P], ident[:]
                        )
                        nc.vector.tensor_copy(vA[:, kb, HD:P], tp[:, 0:HD])
                        nc.vector.tensor_copy(vA[:, kb, P + HD : 2 * P], tp[:, HD:P])

                # ---- attention + out-projection ----
                def attend(t, slot):
                    """Head slot 0/1: qdims at rows [slot*64, slot*64+64)."""
                    off = slot * HD
                    nkb = 4 * (t + 1)
                    ctx = scp.tile([P, QT_W], F32, name="ctx", tag="ctx")
                    q_mv = qT[:, t * QT_W : (t + 1) * QT_W]
                    for g0 in range(0, nkb, GRP):
                        kbs = range(g0, min(g0 + GRP, nkb))
                        gw = len(kbs) * QT_W
                        sc = scp.tile([P, GRP * QT_W], F32, name="sc", tag="sc")
                        for i, kb in enumerate(kbs):
                            # diagonal blocks only need columns r0: onward; the
                            # skipped psum cols hold stale-but-finite garbage
                            # that exp maps to junk p values PV never reads.
                            # The causal mask is folded into the psum
                            # accumulation as identity @ mask (-2000 on the
                            # upper triangle; exp(-2000/8-40) == 0) — keeps
                            # the scores->exp chain PE->ACT with no DVE hop.
                            r = kb * P - t * QT_W
                            r0 = max(0, r)
                            nc.tensor.matmul(
                                sc[:, i * QT_W + r0 : (i + 1) * QT_W],
                                k2[slot][:, kb * P : (kb + 1) * P],
                                q_mv[:, r0:QT_W],
                                start=True,
                                stop=(r < 0),
                            )
                            if r >= 0:
                                nc.tensor.matmul(
                                    sc[:, i * QT_W + r : i * QT_W + r + P],
                                    ident[:],
                                    mask_s[:],
                                    start=False,
                                    stop=True,
                                )
                        if DEBUG and t == 0 and slot == 0 and g0 == 0:
                            dsc = sm.tile([P, QT_W], F32, name="dsc", tag="dsc")
                            nc.vector.tensor_copy(dsc[:], sc[:, 0:QT_W])
                            nc.sync.dma_start(dbg_sc_d[:], dsc[:])
                        # columns before the first block's diagonal were never
                        # written; skip them in the exp too
                        l0 = max(0, kbs[0] * P - t * QT_W)
                        p_t = pt.tile([P, GRP * QT_W], BF16, name="ptile")
                        nc.scalar.activation(
                            p_t[:, l0:gw],
                            sc[:, l0:gw],
                            AF.Exp,
                            scale=0.125,
                            bias=ebias[:],
                        )
                        if DEBUG and t == 0 and slot == 0 and g0 == 0:
                            nc.sync.dma_start(dbg_p_d[:], p_t[:, 0:QT_W])
                        for i, kb in enumerate(kbs):
                            r0 = max(0, kb * P - t * QT_W)
                            nc.tensor.matmul(
                                ctx[:, r0:QT_W],
                                vA[:, kb, slot * P : (slot + 1) * P],
                                p_t[:, i * QT_W + r0 : (i + 1) * QT_W],
                                start=(kb == 0),
                                stop=(kb == nkb - 1),
                            )
                    if DEBUG and t == 0 and slot == 0:
                        dcx = sm.tile([P, QT_W], F32, name="dcx", tag="dcx")
                        nc.vector.tensor_copy(dcx[:], ctx[:])
                        nc.sync.dma_start(dbg_cx_d[:], dcx[:])
                    rr = sm.tile([1, QT_W], F32, name="rr", tag="rr")
                    nc.vector.reciprocal_approx_fast(rr[:], ctx[0:1, :])
                    lb = sm.tile([HD, QT_W], F32, name="lb", tag="lb")
                    nc.gpsimd.partition_broadcast(lb[:], rr[0:1, :])
                    nc.vector.tensor_tensor(
                        cT[off : off + HD, t * QT_W : (t + 1) * QT_W],
                        ctx[HD:P, :],
                        lb[:],
                        mybir.AluOpType.mult,
                    )

                def outproj(t):
                    # each 384-wide f32 half gets its own 1-bank psum tile
                    for b in range(QT_W // P):
                        st = t * 4 + b
                        o_stage = sm.tile([P, D], BF16, name="o_stage", tag="ost")
                        for nch in range(2):
                            po = scp.tile([P, QT_W], F32, name="ctx", tag="ctx")
                            nc.tensor.matmul(
                                po[:, 0 : D // 2],
                                cT[:, st * P : (st + 1) * P],
                                wo_r[:, nch * (D // 2) : (nch + 1) * (D // 2)],
                                start=True,
                                stop=True,
                            )
                            nc.vector.tensor_copy(
                                o_stage[:, nch * (D // 2) : (nch + 1) * (D // 2)],
                                po[:, 0 : D // 2],
                            )
                        nc.sync.dma_start(out_d[st * P : (st + 1) * P, :], o_stage[:])

                # projection first (interleaving proj into the attention loop
                # thrashes the 2-deep sc psum ring and throttles the PE);
                # out-projection trails attention by one tile so its cT
                # dependency (recip/broadcast/mult) never stalls the PE queue
                for t in range(NQT):
                    project(t)
                for t in range(NQT):
                    attend(t, 0)
                    attend(t, 1)
                    if t > 0:
                        outproj(t - 1)
                outproj(NQT - 1)

                if DEBUG:
                    nc.sync.dma_start(dbg_x_d[:], xT[:])
                    nc.sync.dma_start(dbg_q_d[:], qT[:])
                    nc.sync.dma_start(dbg_k0_d[:], k2[0][:])
                    nc.sync.dma_start(dbg_va_d[:], vA[:])
                    nc.sync.dma_start(dbg_ct_d[:], cT[:])

    nc.compile()
    return nc


def _host_inputs(x, W_query, W_key, W_value, W_out):
    mask = np.where(
        np.arange(P)[:, None] <= np.arange(P)[None, :], 0.0, -2000.0
    ).astype(ml_dtypes.bfloat16)
    ident = np.eye(P, dtype=ml_dtypes.bfloat16)
    xt = np.ascontiguousarray(x.T).astype(ml_dtypes.bfloat16)
    in_maps = []
    for core in range(8):
        ha, hb = SLOTS[core]
        sa, sb = SCALES[core]
        ca, cb = slice(ha * HD, (ha + 1) * HD), slice(hb * HD, (hb + 1) * HD)
        in_maps.append(
            {
                "xt": xt,
                "wq": np.ascontiguousarray(
                    np.concatenate([W_query[:, ca], W_query[:, cb]], axis=1)
                ).astype(ml_dtypes.bfloat16),
                "wk": np.ascontiguousarray(
                    np.concatenate([W_key[:, ca], W_key[:, cb]], axis=1)
                ).astype(ml_dtypes.bfloat16),
                "wv": np.ascontiguousarray(
                    np.concatenate([W_value[:, ca], W_value[:, cb]], axis=1)
                ).astype(ml_dtypes.bfloat16),
                "wo": np.ascontiguousarray(
                    np.concatenate([W_out[ca, :] * sa, W_out[cb, :] * sb], axis=0)
                ).astype(ml_dtypes.bfloat16),
                "mask": mask,
                "ident": ident,
            }
        )
    return in_maps


def run(x, W_query, W_key, W_value, W_out, b_out, trace=False):
    global _CACHED_NC
    if _CACHED_NC is None:
        _CACHED_NC = build_nc()
    nc = _CACHED_NC
    in_maps = _host_inputs(x, W_query, W_key, W_value, W_out)
    res = run_bass_kernel_spmd(nc, in_maps, core_ids=list(range(8)), trace=trace)
    out = np.zeros((S, D), dtype=np.float32)
    for core in range(8):
        out += res.results[core]["out"].astype(np.float32)
    out += b_out[None, :].astype(np.float32)
    return out, res


def kernel(x, W_query, W_key, W_value, W_out, b_out):
    x2 = np.asarray(x, dtype=np.float32).reshape(S, D)
    out, _ = run(
        x2,
        np.asarray(W_query, np.float32),
        np.asarray(W_key, np.float32),
        np.asarray(W_value, np.float32),
        np.asarray(W_out, np.float32),
        np.asarray(b_out, np.float32),
    )
    return out.reshape(1, S, D)


# revision 45
# speedup vs baseline: 1.2671x; 1.0090x over previous
"""Causal multi-head attention (B=1, S=4096, D=768, H=12, d_head=64) on 8
Trainium2 NeuronCores.

Sharding: tensor-parallel over heads. 12 heads are mapped onto 16 head-slots
(2 per core); the 4 leftover heads are duplicated onto two slots of the same
core with their W_out rows pre-scaled by 0.5, keeping the SPMD program
uniform across cores. The host sums the 8 partial row-parallel
out-projection outputs and adds b_out.

All matmul operands are bf16 (PSUM accumulates f32): x arrives
host-pre-transposed as xT [768, 4096] bf16 so no on-device transposes or
casts are needed to feed the Q/K/V projections. Q/K/V for one query tile
share a single 3-bank PSUM tile. exp runs on 3-block groups
(ACT instruction overhead amortized) with scale=1/8 and bias=-40 (cancels in
normalization, keeps unnormalized weights in range). Softmax denominators
come free as a ones-column appended to V in the PV stationary; their
reciprocal uses the fast approx DVE op. The out-projection streams per query
tile and the partial output is written in bf16 (halves output DMA).
"""

import sys

sys.path.insert(0, "/opt/trn_rl_repo")

import ml_dtypes
import numpy as np

import concourse.bass as bass
import concourse.tile as tile
from concourse import bacc, mybir
from concourse.bass_utils import run_bass_kernel_spmd

S = 4096
D = 768
HD = 64
P = 128
KC = D // P  # 6 contraction chunks for the projections
QT_W = 512  # query-tile width (one psum bank of f32)
NQT = S // QT_W  # 8 query tiles
NKB = S // P  # 32 key blocks
GRP = 3  # score blocks per exp group (3 psum banks)

F32 = mybir.dt.float32
BF16 = mybir.dt.bfloat16
AF = mybir.ActivationFunctionType
EXP_BIAS = -40.0

SLOTS = [(0, 1), (2, 3), (4, 5), (6, 7), (8, 8), (9, 9), (10, 10), (11, 11)]
SCALES = [(1.0, 1.0)] * 4 + [(0.5, 0.5)] * 4

DEBUG = False  # add DRAM dumps of intermediates (qT/k2/vA/cT)
_CACHED_NC = None


def build_nc():
    nc = bacc.Bacc("TRN2", target_bir_lowering=False, debug=False, num_devices=8)

    xt_d = nc.declare_dram_parameter("xt", [D, S], BF16, isOutput=False)
    wq_d = nc.declare_dram_parameter("wq", [D, P], BF16, isOutput=False)
    wk_d = nc.declare_dram_parameter("wk", [D, P], BF16, isOutput=False)
    wv_d = nc.declare_dram_parameter("wv", [D, P], BF16, isOutput=False)
    wo_d = nc.declare_dram_parameter("wo", [P, D], BF16, isOutput=False)
    mask_d = nc.declare_dram_parameter("mask", [P, P], BF16, isOutput=False)
    ident_d = nc.declare_dram_parameter("ident", [P, P], BF16, isOutput=False)
    out_d = nc.declare_dram_parameter("out", [S, D], BF16, isOutput=True)
    if DEBUG:
        dbg_x_d = nc.declare_dram_parameter("dbg_x", [P, KC, S], BF16, isOutput=True)
        dbg_q_d = nc.declare_dram_parameter("dbg_q", [P, S], BF16, isOutput=True)
        dbg_k0_d = nc.declare_dram_parameter("dbg_k0", [P, S], BF16, isOutput=True)
        dbg_va_d = nc.declare_dram_parameter(
            "dbg_va", [P, NKB, 2 * P], BF16, isOutput=True
        )
        dbg_ct_d = nc.declare_dram_parameter("dbg_ct", [P, S], BF16, isOutput=True)
        dbg_sc_d = nc.declare_dram_parameter("dbg_sc", [P, QT_W], F32, isOutput=True)
        dbg_p_d = nc.declare_dram_parameter("dbg_p", [P, QT_W], BF16, isOutput=True)
        dbg_cx_d = nc.declare_dram_parameter("dbg_cx", [P, QT_W], F32, isOutput=True)

    with tile.TileContext(nc) as tc:
        with (
            tc.tile_pool(name="const", bufs=1) as const,
            tc.tile_pool(name="big", bufs=1) as big,
        ):
            # ---- constants / persistent SBUF ----
            mask_s = const.tile([P, P], BF16)
            nc.sync.dma_start(mask_s[:], mask_d[:])
            ident = const.tile([P, P], BF16)
            nc.sync.dma_start(ident[:], ident_d[:])
            w_r = const.tile([P, KC, 3 * P], BF16)
            nc.sync.dma_start(w_r[:, :, 0:P], wq_d.rearrange("(c p) m -> p c m", p=P))
            nc.sync.dma_start(
                w_r[:, :, P : 2 * P], wk_d.rearrange("(c p) m -> p c m", p=P)
            )
            nc.sync.dma_start(
                w_r[:, :, 2 * P : 3 * P], wv_d.rearrange("(c p) m -> p c m", p=P)
            )
            wo_r = const.tile([P, D], BF16)
            nc.sync.dma_start(wo_r[:], wo_d[:])

            warm = const.tile([P, QT_W], BF16)
            nc.gpsimd.memset(warm[:], 0.5)
            ebias = const.tile([P, 1], F32)
            nc.gpsimd.memset(ebias[:], EXP_BIAS)

            xT = big.tile([P, KC, S], BF16)  # d-on-partitions x, streamed in
            qT = big.tile([P, S], BF16)  # rows 0:64 slot A, 64:128 slot B
            # zero-padded per-slot keys: partial (64-row / 65-col) stationary
            # tiles run the PE at half rate, so keep every stationary 128x128
            k2 = [big.tile([P, S], BF16, name=f"k2_{i}") for i in (0, 1)]
            # vA[key, kb, slot*128+j]: j 0 = ones (denominator lands on psum
            # partition 0, where the custom recip op needs it; psum partition
            # bases must be 32-aligned so ctx values go to partitions 64:128),
            # j 1:64 = 0, j 64:128 = V_slot
            vA = big.tile([P, NKB, 2 * P], BF16)
            cT = big.tile([P, S], BF16)  # normalized ctx: 0:64 A, 64:128 B

            nc.gpsimd.memset(k2[0][HD:P, :], 0.0)
            nc.gpsimd.memset(k2[1][0:HD, :], 0.0)
            for slot in (0, 1):
                nc.gpsimd.memset(vA[:, :, slot * P], 1.0)
                nc.gpsimd.memset(vA[:, :, slot * P + 1 : slot * P + HD], 0.0)

            # one psum pool for the whole kernel: tag "sc" = 2 x 3 banks
            # (proj Q/K/V triple, score groups), tag "ctx" = 2 x 1 bank
            # (V-transposes, ctx accumulators, out-proj halves) -> 8 banks,
            # no phase barrier between projection and attention
            with (
                tc.tile_pool(name="scp", bufs=2, space="PSUM") as scp,
                tc.tile_pool(name="stg", bufs=3) as stg,
                tc.tile_pool(name="pt", bufs=6) as pt,
                tc.tile_pool(name="sm", bufs=6) as sm,
            ):
                # warm the PE p-state while the first x chunk streams in
                for wi in range(16):
                    wps = scp.tile([P, GRP * QT_W], F32, name="sc", tag="sc")
                    nc.tensor.matmul(
                        wps[:, 0:QT_W], ident[:], warm[:], start=True, stop=True
                    )
                def project(t):
                    nc.sync.dma_start(
                        xT[:, :, t * QT_W : (t + 1) * QT_W],
                        xt_d.rearrange("(c p) s -> p c s", p=P)[
                            :, :, t * QT_W : (t + 1) * QT_W
                        ],
                    )
                    pj = scp.tile([P, GRP * QT_W], F32, name="sc", tag="sc")
                    for j in range(3):  # Q, K, V share one 3-bank psum tile
                        for c in range(KC):
                            nc.tensor.matmul(
                                pj[:, j * QT_W : (j + 1) * QT_W],
                                w_r[:, c, j * P : (j + 1) * P],
                                xT[:, c, t * QT_W : (t + 1) * QT_W],
                                start=(c == 0),
                                stop=(c == KC - 1),
                            )
                    # split the four psum->sbuf casts across ACT (idle during
                    # the projection phase) and DVE so the pj ring slot frees
                    # at PE rate
                    nc.scalar.copy(qT[:, t * QT_W : (t + 1) * QT_W], pj[:, 0:QT_W])
                    nc.vector.tensor_copy(
                        k2[0][0:HD, t * QT_W : (t + 1) * QT_W],
                        pj[0:HD, QT_W : 2 * QT_W],
                    )
                    nc.vector.tensor_copy(
                        k2[1][HD:P, t * QT_W : (t + 1) * QT_W],
                        pj[HD:P, QT_W : 2 * QT_W],
                    )
                    vt = stg.tile([P, QT_W], BF16, name="vt", tag="vt")
                    nc.scalar.copy(vt[:], pj[:, 2 * QT_W : 3 * QT_W])
                    for b in range(QT_W // P):
                        kb = t * 4 + b
                        tp = scp.tile([P, P], BF16, name="ctx", tag="ctx")
                        nc.tensor.transpose(
                            tp[:], vt[:, b * P : (b + 1) * P], ident[:]
                        )
                        nc.vector.tensor_copy(vA[:, kb, HD:P], tp[:, 0:HD])
                        nc.vector.tensor_copy(vA[:, kb, P + HD : 2 * P], tp[:, HD:P])

                # ---- attention + out-projection ----
                def attend(t, slot):
                    """Head slot 0/1: qdims at rows [slot*64, slot*64+64)."""
                    off = slot * HD
                    nkb = 4 * (t + 1)
                    ctx = scp.tile([P, QT_W], F32, name="ctx", tag="ctx")
                    q_mv = qT[:, t * QT_W : (t + 1) * QT_W]
                    for g0 in range(0, nkb, GRP):
                        kbs = range(g0, min(g0 + GRP, nkb))
                        gw = len(kbs) * QT_W
                        sc = scp.tile([P, GRP * QT_W], F32, name="sc", tag="sc")
                        for i, kb in enumerate(kbs):
                            # diagonal blocks only need columns r0: onward; the
                            # skipped psum cols hold stale-but-finite garbage
                            # that exp maps to junk p values PV never reads.
                            # The causal mask is folded into the psum
                            # accumulation as identity @ mask (-2000 on the
                            # upper triangle; exp(-2000/8-40) == 0) — keeps
                            # the scores->exp chain PE->ACT with no DVE hop.
                            r = kb * P - t * QT_W
                            r0 = max(0, r)
                            nc.tensor.matmul(
                                sc[:, i * QT_W + r0 : (i + 1) * QT_W],
                                k2[slot][:, kb * P : (kb + 1) * P],
                                q_mv[:, r0:QT_W],
                                start=True,
                                stop=(r < 0),
                            )
                            if r >= 0:
                                nc.tensor.matmul(
                                    sc[:, i * QT_W + r : i * QT_W + r + P],
                                    ident[:],
                                    mask_s[:],
                                    start=False,
                                    stop=True,
                                )
                        if DEBUG and t == 0 and slot == 0 and g0 == 0:
                            dsc = sm.tile([P, QT_W], F32, name="dsc", tag="dsc")
                            nc.vector.tensor_copy(dsc[:], sc[:, 0:QT_W])
                            nc.sync.dma_start(dbg_sc_d[:], dsc[:])
                        # columns before the first block's diagonal were never
                        # written; skip them in the exp too
                        l0 = max(0, kbs[0] * P - t * QT_W)
                        p_t = pt.tile([P, GRP * QT_W], BF16, name="ptile")
                        nc.scalar.activation(
                            p_t[:, l0:gw],
                            sc[:, l0:gw],
                            AF.Exp,
                            scale=0.125,
                            bias=ebias[:],
                        )
                        if DEBUG and t == 0 and slot == 0 and g0 == 0:
                            nc.sync.dma_start(dbg_p_d[:], p_t[:, 0:QT_W])
                        for i, kb in enumerate(kbs):
                            r0 = max(0, kb * P - t * QT_W)
                            nc.tensor.matmul(
                                ctx[:, r0:QT_W],
                                vA[:, kb, slot * P : (slot + 1) * P],
                                p_t[:, i * QT_W + r0 : (i + 1) * QT_W],
                                start=(kb == 0),
                                stop=(kb == nkb - 1),
                            )
                    if DEBUG and t == 0 and slot == 0:
                        dcx = sm.tile([P, QT_W], F32, name="dcx", tag="dcx")
                        nc.vector.tensor_copy(dcx[:], ctx[:])
                        nc.sync.dma_start(dbg_cx_d[:], dcx[:])
                    rr = sm.tile([1, QT_W], F32, name="rr", tag="rr")
                    nc.vector.reciprocal_approx_fast(rr[:], ctx[0:1, :])
                    lb = sm.tile([HD, QT_W], F32, name="lb", tag="lb")
                    nc.gpsimd.partition_broadcast(lb[:], rr[0:1, :])
                    nc.vector.tensor_tensor(
                        cT[off : off + HD, t * QT_W : (t + 1) * QT_W],
                        ctx[HD:P, :],
                        lb[:],
                        mybir.AluOpType.mult,
                    )

                def outproj(t):
                    # each 384-wide f32 half gets its own 1-bank psum tile
                    for b in range(QT_W // P):
                        st = t * 4 + b
                        o_stage = sm.tile([P, D], BF16, name="o_stage", tag="ost")
                        for nch in range(2):
                            po = scp.tile([P, QT_W], F32, name="ctx", tag="ctx")
                            nc.tensor.matmul(
                                po[:, 0 : D // 2],
                                cT[:, st * P : (st + 1) * P],
                                wo_r[:, nch * (D // 2) : (nch + 1) * (D // 2)],
                                start=True,
                                stop=True,
                            )
                            nc.vector.tensor_copy(
                                o_stage[:, nch * (D // 2) : (nch + 1) * (D // 2)],
                                po[:, 0 : D // 2],
                            )
                        nc.sync.dma_start(out_d[st * P : (st + 1) * P, :], o_stage[:])

                # projection first (interleaving proj into the attention loop
                # thrashes the 2-deep sc psum ring and throttles the PE);
                # out-projection trails attention by one tile so its cT
                # dependency (recip/broadcast/mult) never stalls the PE queue
                for t in range(NQT):
                    project(t)
                for t in range(NQT):
                    attend(t, 0)
                    attend(t, 1)
                    if t > 0:
                        outproj(t - 1)
                outproj(NQT - 1)

                if DEBUG:
                    nc.sync.dma_start(dbg_x_d[:], xT[:])
                    nc.sync.dma_start(dbg_q_d[:], qT[:])
                    nc.sync.dma_start(dbg_k0_d[:], k2[0][:])
                    nc.sync.dma_start(dbg_va_d[:], vA[:])
                    nc.sync.dma_start(dbg_ct_d[:], cT[:])

    nc.compile()
    return nc


def _host_inputs(x, W_query, W_key, W_value, W_out):
    mask = np.where(
        np.arange(P)[:, None] <= np.arange(P)[None, :], 0.0, -2000.0
    ).astype(ml_dtypes.bfloat16)
    ident = np.eye(P, dtype=ml_dtypes.bfloat16)
    xt = np.ascontiguousarray(x.T).astype(ml_dtypes.bfloat16)
    in_maps = []
    for core in range(8):
        ha, hb = SLOTS[core]
        sa, sb = SCALES[core]
        ca, cb = slice(ha * HD, (ha + 1) * HD), slice(hb * HD, (hb + 1) * HD)
        in_maps.append(
            {
                "xt": xt,
                "wq": np.ascontiguousarray(
                    np.concatenate([W_query[:, ca], W_query[:, cb]], axis=1)
                ).astype(ml_dtypes.bfloat16),
                "wk": np.ascontiguousarray(
                    np.concatenate([W_key[:, ca], W_key[:, cb]], axis=1)
                ).astype(ml_dtypes.bfloat16),
                "wv": np.ascontiguousarray(
                    np.concatenate([W_value[:, ca], W_value[:, cb]], axis=1)
                ).astype(ml_dtypes.bfloat16),
                "wo": np.ascontiguousarray(
                    np.concatenate([W_out[ca, :] * sa, W_out[cb, :] * sb], axis=0)
                ).astype(ml_dtypes.bfloat16),
                "mask": mask,
                "ident": ident,
            }
        )
    return in_maps


def run(x, W_query, W_key, W_value, W_out, b_out, trace=False):
    global _CACHED_NC
    if _CACHED_NC is None:
        _CACHED_NC = build_nc()
    nc = _CACHED_NC
    in_maps = _host_inputs(x, W_query, W_key, W_value, W_out)
    res = run_bass_kernel_spmd(nc, in_maps, core_ids=list(range(8)), trace=trace)
    out = np.zeros((S, D), dtype=np.float32)
    for core in range(8):
        out += res.results[core]["out"].astype(np.float32)
    out += b_out[None, :].astype(np.float32)
    return out, res


def kernel(x, W_query, W_key, W_value, W_out, b_out):
    x2 = np.asarray(x, dtype=np.float32).reshape(S, D)
    out, _ = run(
        x2,
        np.asarray(W_query, np.float32),
        np.asarray(W_key, np.float32),
        np.asarray(W_value, np.float32),
        np.asarray(W_out, np.float32),
        np.asarray(b_out, np.float32),
    )
    return out.reshape(1, S, D)
